# revision 28
# baseline (speedup 1.0000x reference)
"""Causal multi-head attention (B=4, N=2048, D=1024, H=16, Dh=64) on 8 TRN2 cores.

Sharding: core c handles batch b=c//2 and head-group g=c%2 (8 of 16 heads).
Megatron-style: Wq/Wkv column-parallel, Wo row-parallel; the per-pair partial
outputs are combined with a ReduceScatter(add) over core pairs {2b, 2b+1}.

Everything on-device runs in a transposed layout ([feature, token]) so that no
PE transposes are needed anywhere:
  Qt/Kt = W-stationary matmuls of xT            -> [inner, tok]
  S^T   = Kt-stationary, Qt-moving              -> [key, query]  (2 heads row-packed,
                                                   concurrently via PE row-tiling)
  P^T   = exp(scale*S^T) via ACT, 0/1-masked    -> [key, query]  bf16
  O^T   = V'-stationary ([V | ones]), P^T-moving-> [65, query]   (row 64 = softmax denom)
  out^T = Wo-stationary, O^T-moving             -> [dmodel, tok]
The host transposes x and pre-casts x/W to bf16; partial outputs travel as
bf16 through the ReduceScatter and are upcast to f32 on the way out.

The pipeline is interleaved per query-span so projections, attention, and the
output projection of successive spans overlap, keeping the PE array dense
(HAM stays at K=8/8).
"""

import sys

sys.path.insert(0, "/opt/trn_rl_repo")

import numpy as np

import concourse.bass as bass
import concourse.mybir as mybir
from concourse import bacc, tile
from concourse.bass_utils import run_bass_kernel_spmd

F32 = mybir.dt.float32
BF16 = mybir.dt.bfloat16

B = 4
N = 2048
DM = 1024          # d_model
H = 16
DH = 64
HL = 8             # local heads per core
IL = HL * DH       # 512, local inner dim
SCALE = DH ** -0.5
SPAN = 512         # query-span / matmul moving size
NSP = N // SPAN    # 4
NKB = N // 128     # 16 key/token blocks
NCORES = 8
INTERLEAVE = True


def build_program(repeat=1, for_sim=False):
    nc = bacc.Bacc("TRN2", target_bir_lowering=False, debug=False,
                   num_devices=1 if for_sim else NCORES)

    xT_d = nc.dram_tensor("xT", [DM, N], BF16, kind="ExternalInput").ap()
    wq_d = nc.dram_tensor("wq", [DM, IL], BF16, kind="ExternalInput").ap()
    wk_d = nc.dram_tensor("wk", [DM, IL], BF16, kind="ExternalInput").ap()
    wv_d = nc.dram_tensor("wv", [DM, IL], BF16, kind="ExternalInput").ap()
    wo_d = nc.dram_tensor("wo", [IL, DM], BF16, kind="ExternalInput").ap()
    bias_d = nc.dram_tensor("bias", [DM, 1], F32, kind="ExternalInput").ap()
    out_d = nc.dram_tensor("out", [DM // 2, N], BF16, kind="ExternalOutput").ap()

    with tile.TileContext(nc) as tc:
        with (
            tc.tile_pool(name="weights", bufs=1) as wpool,
            tc.tile_pool(name="acts", bufs=1) as apool,
            tc.tile_pool(name="work", bufs=3) as work,
            tc.tile_pool(name="psum", bufs=1, space="PSUM") as psum,
            tc.tile_pool(name="dram", bufs=1, space="DRAM") as dram,
        ):
            # ---------- stage 0: load (already bf16 from host) ----------
            xh = [apool.tile([128, N], BF16, name=f"xh{pb}", tag=f"xh{pb}")
                  for pb in range(DM // 128)]

            def load_w(src, n_pb, ncols, nm):
                # one big DMA into a [128, n_pb, ncols] tile; per-block
                # [128, ncols] views are sliced out for the matmuls
                t = wpool.tile([128, n_pb, ncols], BF16, name=nm, tag=nm)
                nc.sync.dma_start(
                    t[:], src[:].rearrange("(b p) c -> p b c", p=128))
                return [t[:, pb, :] for pb in range(n_pb)]

            # interleave x-span0 / wq blocks so the first projection's
            # accumulation chain starts after only a few transfers
            wqh_t = wpool.tile([128, DM // 128, IL], BF16, name="wqh",
                               tag="wqh")
            wqh = [wqh_t[:, pb, :] for pb in range(DM // 128)]
            for pb in range(DM // 128):
                nc.sync.dma_start(xh[pb][:, 0:SPAN],
                                  xT_d[pb * 128:(pb + 1) * 128, 0:SPAN])
                nc.sync.dma_start(wqh_t[:, pb, :],
                                  wq_d[pb * 128:(pb + 1) * 128, :])
            wkh = load_w(wk_d, DM // 128, IL, "wkh")
            wvh = load_w(wv_d, DM // 128, IL, "wvh")
            # rest of x in one bulk DMA per partition block
            for pb in range(DM // 128):
                nc.sync.dma_start(xh[pb][:, SPAN:N],
                                  xT_d[pb * 128:(pb + 1) * 128, SPAN:N])
            woh = load_w(wo_d, IL // 128, DM, "woh")

            bias_sb = wpool.tile([128, DM // 128], F32, name="bias_sb")
            for mb in range(DM // 128):
                nc.sync.dma_start(bias_sb[:, mb:mb + 1],
                                  bias_d[mb * 128:(mb + 1) * 128, :])

            # 0/1 lower-triangle mask (keep query >= key within a diag block)
            tri_f = work.tile([128, 128], F32, tag="tri_f", bufs=1)
            nc.gpsimd.memset(tri_f[:], 1.0)
            nc.gpsimd.affine_select(
                out=tri_f[:], in_=tri_f[:],
                compare_op=mybir.AluOpType.is_ge,
                fill=0.0, base=0, channel_multiplier=-1,
                pattern=[[1, 128]],
            )
            tri01 = wpool.tile([128, 1, 128], BF16, name="tri01")
            nc.vector.tensor_copy(tri01[:, 0, :], tri_f[:])
            ones64 = wpool.tile([1, DH], F32, name="ones64")
            nc.vector.memset(ones64[:], 1.0)

            # PE warm-up: ~6us of dummy matmuls overlapping the input DMAs
            # so the HAM clock gate is at K=8/8 when the first real
            # projection matmul issues (cold PE runs at half rate)
            warm_ps = psum.tile([128, 128], F32, tag="projrb", bufs=2,
                                name="warm_ps")
            for _w in range(60):
                nc.tensor.matmul(warm_ps[:], tri01[:, 0, :], tri01[:, 0, :],
                                 start=True, stop=True)

            for _rep in range(repeat):
                # per-span activation tiles
                qth = [[apool.tile([128, SPAN], BF16, name=f"qt{pb}_{sp}",
                                   tag=f"qt{pb}_{sp}")
                        for sp in range(NSP)] for pb in range(IL // 128)]
                kth = [[apool.tile([128, SPAN], BF16, name=f"kt{pb}_{sp}",
                                   tag=f"kt{pb}_{sp}")
                        for sp in range(NSP)] for pb in range(IL // 128)]
                vth = [apool.tile([128, HL, DH + 1], BF16, name=f"vt{tb}",
                                  tag=f"vt{tb}") for tb in range(NKB)]
                oth = [[apool.tile([128, SPAN], BF16, name=f"ot{pb}_{sp}",
                                   tag=f"ot{pb}_{sp}")
                        for sp in range(NSP)] for pb in range(IL // 128)]
                parts = [dram.tile([DM, SPAN], BF16, name=f"part{sp}")
                         for sp in range(NSP)]

                def qk_group(wt, dst, pb, sp):
                    def go():
                        pp = psum.tile([128, SPAN], F32, tag="projrb",
                                       bufs=2, name="pp")
                        for kk in range(DM // 128):
                            nc.tensor.matmul(
                                pp[:],
                                wt[kk][:, pb * 128:(pb + 1) * 128],
                                xh[kk][:, sp * SPAN:(sp + 1) * SPAN],
                                start=(kk == 0), stop=(kk == DM // 128 - 1),
                            )
                        nc.vector.tensor_copy(dst[pb][sp][:], pp[:])
                    return go

                def v_group(tb):
                    def go():
                        pp = psum.tile([128, IL], F32, tag="projrb", bufs=2,
                                       name="ppv")
                        for kk in range(DM // 128):
                            nc.tensor.matmul(
                                pp[:], xh[kk][:, tb * 128:(tb + 1) * 128],
                                wvh[kk][:],
                                start=(kk == 0), stop=(kk == DM // 128 - 1),
                            )
                        nc.vector.tensor_copy(
                            vth[tb][:, :, 0:DH],
                            pp[:].rearrange("p (h d) -> p h d", h=HL))
                        nc.vector.memset(vth[tb][:, :, DH:DH + 1], 1.0)
                    return go

                def wo_group(mb, sp):
                    def go():
                        # the last span's groups borrow the (idle by then)
                        # sT slots so four groups can pre-run their first
                        # three matmuls while the final attn_norm finishes
                        tag = "sT" if sp == NSP - 1 and mb % 2 == 0 \
                            else "projrb"
                        pw = psum.tile([128, SPAN], F32, tag=tag, bufs=2,
                                       name="pw")
                        for ib in range(IL // 128):
                            nc.tensor.matmul(
                                pw[:],
                                woh[ib][:, mb * 128:(mb + 1) * 128],
                                oth[ib][sp][:],
                                start=(ib == 0), stop=(ib == IL // 128 - 1),
                            )
                        po = work.tile([128, SPAN], BF16, tag="po", bufs=3,
                                       name="po")
                        nc.vector.tensor_scalar(
                            out=po[:], in0=pw[:],
                            scalar1=bias_sb[:, mb:mb + 1], scalar2=None,
                            op0=mybir.AluOpType.add,
                        )
                        nc.sync.dma_start(
                            parts[sp][mb * 128:(mb + 1) * 128, :], po[:])
                    return go

                rs_tiles = {}

                def rs_trigger(sp):
                    if for_sim:
                        rs_tiles[sp] = parts[sp]
                        return
                    rs = dram.tile([DM // 2, SPAN], BF16, name=f"rs{sp}")
                    nc.gpsimd.collective_compute(
                        "ReduceScatter", mybir.AluOpType.add,
                        replica_groups=[[0, 1], [2, 3], [4, 5], [6, 7]],
                        ins=[parts[sp].opt()], outs=[rs.opt()],
                    )
                    rs_tiles[sp] = rs

                def rs_out(sp):
                    # issued well after the trigger so the Sync queue never
                    # blocks on the collective semaphore
                    nc.sync.dma_start(out_d[:, sp * SPAN:(sp + 1) * SPAN],
                                      rs_tiles[sp][0:DM // 2, :])



                def proj_tasks(sp, split_jit=False):
                    """Returns (immediate, deferred) task lists. When
                    split_jit, tasks whose outputs are consumed late in
                    span sp's own attention (late key-spans / late head
                    pairs) are deferred into that span, in deadline order,
                    so the PE has filler work while ACT paces the steps."""
                    qk = lambda wt, dst, pb: qk_group(wt, dst, pb, sp)
                    if not split_jit:
                        tasks = [qk(wqh, qth, pb) for pb in range(IL // 128)]
                        tasks += [qk(wkh, kth, pb) for pb in range(IL // 128)]
                        tasks += [v_group(tb)
                                  for tb in range(4 * sp, 4 * sp + 4)]
                        return tasks, []
                    immediate = [qk(wqh, qth, 0), qk(wqh, qth, 1)]
                    deferred = [
                        qk(wkh, kth, 0),
                        v_group(4 * sp), v_group(4 * sp + 1),
                        v_group(4 * sp + 2), v_group(4 * sp + 3),
                        qk(wkh, kth, 1), qk(wqh, qth, 2),
                        qk(wkh, kth, 2), qk(wqh, qth, 3),
                        qk(wkh, kth, 3),
                    ]
                    return immediate, deferred

                def attn_step(hp, qs, kb, o_ps, nkb):
                    off = kb * 128 - qs * SPAN   # <0 for off-diag
                    lo = max(off, 0)             # first causal query
                    sg = psum.tile([128, 2, SPAN], F32, tag="sT",
                                   bufs=2, name="sg")
                    for i in range(2):
                        nc.tensor.matmul(
                            sg[:, i, lo:SPAN],
                            kth[hp][kb // 4][64 * i:64 * i + 64,
                                             (kb % 4) * 128:
                                             (kb % 4) * 128 + 128],
                            qth[hp][qs][64 * i:64 * i + 64, lo:SPAN],
                            start=True, stop=True,
                        )
                    pt = work.tile([128, 2, SPAN], BF16, tag="pT",
                                   bufs=6, name="pt")
                    nc.scalar.activation(
                        pt[:, :, lo:SPAN], sg[:, :, lo:SPAN],
                        mybir.ActivationFunctionType.Exp, scale=SCALE)
                    if off >= 0:
                        # zero the strictly-upper triangle of the
                        # diagonal 128x128 block for both heads at once
                        nc.vector.tensor_tensor(
                            out=pt[:, :, lo:lo + 128],
                            in0=pt[:, :, lo:lo + 128],
                            in1=tri01[:].broadcast_to([128, 2, 128]),
                            op=mybir.AluOpType.mult,
                        )
                    for i in range(2):
                        nc.tensor.matmul(
                            o_ps[:, i, lo:SPAN],
                            vth[kb][:, 2 * hp + i, :],
                            pt[:, i, lo:SPAN],
                            start=(kb == 0), stop=(kb == nkb - 1),
                        )

                def attn_norm(hp, qs, o_ps, pe_bcast=False):
                    # one copy frees both o_ps PSUM banks for the next hp
                    o_sb = work.tile([DH + 1, 2, SPAN], F32, tag="o_sb",
                                     bufs=2, name="o_sb")
                    nc.vector.tensor_copy(o_sb[:], o_ps[:])
                    r0 = work.tile([1, 2, SPAN], F32, tag="r0",
                                   bufs=2, name="r0")
                    nc.vector.tensor_copy(r0[0:1, :, :],
                                          o_sb[DH:DH + 1, :, :])
                    rcp = work.tile([1, 2, SPAN], F32, tag="rcp",
                                    bufs=2, name="rcp")
                    nc.vector.reciprocal_approx_fast(rcp[:], r0[0:1, :, :])
                    if pe_bcast:
                        # final norm of the kernel: broadcast 1/denom via a
                        # K=1 matmul into the (idle) o_ps slot — shorter
                        # critical chain than the gpsimd partition_broadcast
                        rb_ps = psum.tile([DH, 2, SPAN], F32, tag="oT",
                                          bufs=1, name="rb_ps")
                        for i in range(2):
                            nc.tensor.matmul(rb_ps[:, i, :], ones64[:],
                                             rcp[0:1, i, :],
                                             start=True, stop=True)
                        rb = rb_ps
                    else:
                        rbt = work.tile([64, 2, SPAN], F32, tag="rb",
                                        bufs=2, name="rb")
                        nc.gpsimd.partition_broadcast(rbt[:], rcp[0:1, :, :])
                        rb = rbt
                    for i in range(2):
                        nc.vector.tensor_tensor(
                            out=oth[hp][qs][64 * i:64 * i + 64, :],
                            in0=o_sb[0:DH, i, :], in1=rb[:, i, :],
                            op=mybir.AluOpType.mult,
                        )

                # prologue: projections for span 0
                for t in proj_tasks(0)[0]:
                    t()

                carry = []
                for sp in range(NSP):
                    qs = sp
                    nkb = 4 * qs + 4
                    # independent PE work to weave into attention stalls:
                    # this span's deferred projections (deadline-ordered),
                    # next span's projections + previous span's out-proj
                    pending = list(carry)
                    carry = []
                    if sp + 1 < NSP:
                        imm, carry = proj_tasks(
                            sp + 1, split_jit=(sp + 1 == NSP - 1))
                        pending += imm
                    if sp >= 1:
                        pending += [wo_group(mb, sp - 1)
                                    for mb in range(DM // 128)]
                        pending += [lambda sp=sp: rs_trigger(sp - 1)]
                    nsteps = nkb * (HL // 2)
                    stride = max(1, nsteps // max(1, len(pending)))
                    step = 0
                    for hp in range(HL // 2):
                        o_ps = psum.tile([DH + 1, 2, SPAN], F32, tag="oT",
                                         bufs=1, name="o_ps")
                        for kb in range(nkb):
                            attn_step(hp, qs, kb, o_ps, nkb)
                            step += 1
                            if INTERLEAVE and step % stride == 0 and pending:
                                pending.pop(0)()
                        attn_norm(hp, qs, o_ps,
                                  pe_bcast=(sp == NSP - 1 and
                                            hp == HL // 2 - 1))
                    while pending:
                        pending.pop(0)()
                # epilogue: out-projection of the last span
                # epilogue: all rs->out copies happen here, where the
                # collectives for spans 0..NSP-2 have long completed, so no
                # DMA queue ever blocks mid-pipeline on a collective sem
                for mb in range(DM // 128):
                    wo_group(mb, NSP - 1)()
                rs_trigger(NSP - 1)
                for sp in range(NSP):
                    rs_out(sp)

    nc.compile()
    return nc


_program_cache = None


def make_in_maps(inputs):
    from ml_dtypes import bfloat16

    x = np.asarray(inputs["x"], dtype=np.float32)
    Wq = np.asarray(inputs["Wq"], dtype=np.float32)
    Wkv = np.asarray(inputs["Wkv"], dtype=np.float32)
    Wo = np.asarray(inputs["Wo"], dtype=np.float32)
    bo = np.asarray(inputs["bo"], dtype=np.float32)
    xTb = [np.ascontiguousarray(x[b].T).astype(bfloat16) for b in range(B)]
    wqb = [np.ascontiguousarray(Wq[:, g * IL:(g + 1) * IL]).astype(bfloat16)
           for g in range(2)]
    wkb = [np.ascontiguousarray(
               Wkv[:, g * IL:(g + 1) * IL]).astype(bfloat16)
           for g in range(2)]
    wvb = [np.ascontiguousarray(
               Wkv[:, DM + g * IL:DM + (g + 1) * IL]).astype(bfloat16)
           for g in range(2)]
    wob = [np.ascontiguousarray(Wo[g * IL:(g + 1) * IL, :]).astype(bfloat16)
           for g in range(2)]
    bias0 = bo.reshape(DM, 1)
    bias1 = np.zeros_like(bias0)
    in_maps = []
    for c in range(NCORES):
        b, g = c // 2, c % 2
        in_maps.append({
            "xT": xTb[b],
            "wq": wqb[g],
            "wk": wkb[g],
            "wv": wvb[g],
            "wo": wob[g],
            "bias": bias0 if g == 0 else bias1,
        })
    return in_maps


def kernel(x, Wq, Wkv, Wo, bo):
    global _program_cache
    if _program_cache is None:
        _program_cache = build_program()
    nc = _program_cache

    in_maps = make_in_maps(dict(x=x, Wq=Wq, Wkv=Wkv, Wo=Wo, bo=bo))
    res = run_bass_kernel_spmd(nc, in_maps, list(range(NCORES)))

    out = np.empty((B, N, DM), dtype=np.float32)
    for b in range(B):
        top = res.results[2 * b]["out"]       # dmodel rows 0:512 (bf16)
        bot = res.results[2 * b + 1]["out"]   # dmodel rows 512:1024 (bf16)
        out[b] = np.concatenate([top, bot], axis=0).T.astype(np.float32)
    return out


# revision 32
# speedup vs baseline: 1.0235x; 1.0235x over previous
"""Causal multi-head attention (B=4, N=2048, D=1024, H=16, Dh=64) on 8 TRN2 cores.

Sharding: core c handles batch b=c//2 and head-group g=c%2 (8 of 16 heads).
Megatron-style: Wq/Wkv column-parallel, Wo row-parallel; the per-pair partial
outputs are combined with a ReduceScatter(add) over core pairs {2b, 2b+1}.

Everything on-device runs in a transposed layout ([feature, token]) so that no
PE transposes are needed anywhere:
  Qt/Kt = W-stationary matmuls of xT            -> [inner, tok]
  S^T   = Kt-stationary, Qt-moving              -> [key, query]  (2 heads row-packed,
                                                   concurrently via PE row-tiling)
  P^T   = exp(scale*S^T) via ACT, 0/1-masked    -> [key, query]  bf16
  O^T   = V'-stationary ([V | ones]), P^T-moving-> [65, query]   (row 64 = softmax denom)
  out^T = Wo-stationary, O^T-moving             -> [dmodel, tok]
The host transposes x and pre-casts x/W to bf16; partial outputs travel as
bf16 through the ReduceScatter and are upcast to f32 on the way out.

The pipeline is interleaved per query-span so projections, attention, and the
output projection of successive spans overlap, keeping the PE array dense
(HAM stays at K=8/8).
"""

import sys

sys.path.insert(0, "/opt/trn_rl_repo")

import numpy as np

import concourse.bass as bass
import concourse.mybir as mybir
from concourse import bacc, tile
from concourse.bass_utils import run_bass_kernel_spmd

F32 = mybir.dt.float32
BF16 = mybir.dt.bfloat16

B = 4
N = 2048
DM = 1024          # d_model
H = 16
DH = 64
HL = 8             # local heads per core
IL = HL * DH       # 512, local inner dim
SCALE = DH ** -0.5
SPAN = 512         # query-span / matmul moving size
NSP = N // SPAN    # 4
NKB = N // 128     # 16 key/token blocks
NCORES = 8
INTERLEAVE = True


def build_program(repeat=1, for_sim=False):
    nc = bacc.Bacc("TRN2", target_bir_lowering=False, debug=False,
                   num_devices=1 if for_sim else NCORES)

    xT_d = nc.dram_tensor("xT", [DM, N], BF16, kind="ExternalInput").ap()
    wq_d = nc.dram_tensor("wq", [DM, IL], BF16, kind="ExternalInput").ap()
    wk_d = nc.dram_tensor("wk", [DM, IL], BF16, kind="ExternalInput").ap()
    wv_d = nc.dram_tensor("wv", [DM, IL], BF16, kind="ExternalInput").ap()
    wo_d = nc.dram_tensor("wo", [IL, DM], BF16, kind="ExternalInput").ap()
    bias_d = nc.dram_tensor("bias", [DM, 1], F32, kind="ExternalInput").ap()
    out_d = nc.dram_tensor("out", [DM // 2, N], BF16, kind="ExternalOutput").ap()

    with tile.TileContext(nc) as tc:
        with (
            tc.tile_pool(name="weights", bufs=1) as wpool,
            tc.tile_pool(name="acts", bufs=1) as apool,
            tc.tile_pool(name="work", bufs=3) as work,
            tc.tile_pool(name="psum", bufs=1, space="PSUM") as psum,
            tc.tile_pool(name="dram", bufs=1, space="DRAM") as dram,
        ):
            # ---------- stage 0: load (already bf16 from host) ----------
            xh = [apool.tile([128, N], BF16, name=f"xh{pb}", tag=f"xh{pb}")
                  for pb in range(DM // 128)]

            def load_w(src, n_pb, ncols, nm):
                # one big DMA into a [128, n_pb, ncols] tile; per-block
                # [128, ncols] views are sliced out for the matmuls
                t = wpool.tile([128, n_pb, ncols], BF16, name=nm, tag=nm)
                nc.sync.dma_start(
                    t[:], src[:].rearrange("(b p) c -> p b c", p=128))
                return [t[:, pb, :] for pb in range(n_pb)]

            # interleave x-span0 / wq blocks so the first projection's
            # accumulation chain starts after only a few transfers
            wqh_t = wpool.tile([128, DM // 128, IL], BF16, name="wqh",
                               tag="wqh")
            wqh = [wqh_t[:, pb, :] for pb in range(DM // 128)]
            for pb in range(DM // 128):
                nc.sync.dma_start(xh[pb][:, 0:SPAN],
                                  xT_d[pb * 128:(pb + 1) * 128, 0:SPAN])
                nc.sync.dma_start(wqh_t[:, pb, :],
                                  wq_d[pb * 128:(pb + 1) * 128, :])
            wkh = load_w(wk_d, DM // 128, IL, "wkh")
            wvh = load_w(wv_d, DM // 128, IL, "wvh")
            # rest of x in one bulk DMA per partition block
            for pb in range(DM // 128):
                nc.sync.dma_start(xh[pb][:, SPAN:N],
                                  xT_d[pb * 128:(pb + 1) * 128, SPAN:N])
            woh = load_w(wo_d, IL // 128, DM, "woh")

            bias_sb = wpool.tile([128, DM // 128], F32, name="bias_sb")
            for mb in range(DM // 128):
                nc.sync.dma_start(bias_sb[:, mb:mb + 1],
                                  bias_d[mb * 128:(mb + 1) * 128, :])

            # 0/1 lower-triangle mask (keep query >= key within a diag block)
            tri_f = work.tile([128, 128], F32, tag="tri_f", bufs=1)
            nc.gpsimd.memset(tri_f[:], 1.0)
            nc.gpsimd.affine_select(
                out=tri_f[:], in_=tri_f[:],
                compare_op=mybir.AluOpType.is_ge,
                fill=0.0, base=0, channel_multiplier=-1,
                pattern=[[1, 128]],
            )
            tri01 = wpool.tile([128, 1, 128], BF16, name="tri01")
            nc.vector.tensor_copy(tri01[:, 0, :], tri_f[:])
            ones64 = wpool.tile([1, DH], F32, name="ones64")
            nc.vector.memset(ones64[:], 1.0)

            # PE warm-up: ~6us of dummy matmuls overlapping the input DMAs
            # so the HAM clock gate is at K=8/8 when the first real
            # projection matmul issues (cold PE runs at half rate)
            warm_ps = psum.tile([128, 128], F32, tag="projrb", bufs=2,
                                name="warm_ps")
            for _w in range(60):
                nc.tensor.matmul(warm_ps[:], tri01[:, 0, :], tri01[:, 0, :],
                                 start=True, stop=True)

            for _rep in range(repeat):
                # per-span activation tiles
                qth = [[apool.tile([128, SPAN], BF16, name=f"qt{pb}_{sp}",
                                   tag=f"qt{pb}_{sp}")
                        for sp in range(NSP)] for pb in range(IL // 128)]
                kth = [[apool.tile([128, SPAN], BF16, name=f"kt{pb}_{sp}",
                                   tag=f"kt{pb}_{sp}")
                        for sp in range(NSP)] for pb in range(IL // 128)]
                vth = [apool.tile([128, HL, DH + 1], BF16, name=f"vt{tb}",
                                  tag=f"vt{tb}") for tb in range(NKB)]
                oth = [[apool.tile([128, SPAN], BF16, name=f"ot{pb}_{sp}",
                                   tag=f"ot{pb}_{sp}")
                        for sp in range(NSP)] for pb in range(IL // 128)]
                parts = [dram.tile([DM, SPAN], BF16, name=f"part{sp}")
                         for sp in range(NSP)]

                def qk_group(wt, dst, pb, sp):
                    def go():
                        pp = psum.tile([128, SPAN], F32, tag="projrb",
                                       bufs=2, name="pp")
                        for kk in range(DM // 128):
                            nc.tensor.matmul(
                                pp[:],
                                wt[kk][:, pb * 128:(pb + 1) * 128],
                                xh[kk][:, sp * SPAN:(sp + 1) * SPAN],
                                start=(kk == 0), stop=(kk == DM // 128 - 1),
                            )
                        nc.vector.tensor_copy(dst[pb][sp][:], pp[:])
                    return go

                def v_group(tb):
                    def go():
                        pp = psum.tile([128, IL], F32, tag="projrb", bufs=2,
                                       name="ppv")
                        for kk in range(DM // 128):
                            nc.tensor.matmul(
                                pp[:], xh[kk][:, tb * 128:(tb + 1) * 128],
                                wvh[kk][:],
                                start=(kk == 0), stop=(kk == DM // 128 - 1),
                            )
                        nc.vector.tensor_copy(
                            vth[tb][:, :, 0:DH],
                            pp[:].rearrange("p (h d) -> p h d", h=HL))
                        nc.vector.memset(vth[tb][:, :, DH:DH + 1], 1.0)
                    return go

                def wo_group(mb, sp):
                    def go():
                        # the last span's groups borrow the (idle by then)
                        # sT slots so four groups can pre-run their first
                        # three matmuls while the final attn_norm finishes
                        tag = "sT" if sp == NSP - 1 and mb % 2 == 0 \
                            else "projrb"
                        pw = psum.tile([128, SPAN], F32, tag=tag, bufs=2,
                                       name="pw")
                        for ib in range(IL // 128):
                            nc.tensor.matmul(
                                pw[:],
                                woh[ib][:, mb * 128:(mb + 1) * 128],
                                oth[ib][sp][:],
                                start=(ib == 0), stop=(ib == IL // 128 - 1),
                            )
                        po = work.tile([128, SPAN], BF16, tag="po", bufs=3,
                                       name="po")
                        nc.vector.tensor_scalar(
                            out=po[:], in0=pw[:],
                            scalar1=bias_sb[:, mb:mb + 1], scalar2=None,
                            op0=mybir.AluOpType.add,
                        )
                        nc.sync.dma_start(
                            parts[sp][mb * 128:(mb + 1) * 128, :], po[:])
                    return go

                rs_tiles = {}

                def rs_trigger(sp):
                    if for_sim:
                        rs_tiles[sp] = parts[sp]
                        return
                    rs = dram.tile([DM // 2, SPAN], BF16, name=f"rs{sp}")
                    nc.gpsimd.collective_compute(
                        "ReduceScatter", mybir.AluOpType.add,
                        replica_groups=[[0, 1], [2, 3], [4, 5], [6, 7]],
                        ins=[parts[sp].opt()], outs=[rs.opt()],
                    )
                    rs_tiles[sp] = rs

                def rs_out(sp):
                    # issued well after the trigger so the Sync queue never
                    # blocks on the collective semaphore
                    nc.sync.dma_start(out_d[:, sp * SPAN:(sp + 1) * SPAN],
                                      rs_tiles[sp][0:DM // 2, :])



                def proj_tasks(sp, split_jit=False):
                    """Returns (immediate, deferred) task lists. When
                    split_jit, tasks whose outputs are consumed late in
                    span sp's own attention (late key-spans / late head
                    pairs) are deferred into that span, in deadline order,
                    so the PE has filler work while ACT paces the steps."""
                    qk = lambda wt, dst, pb: qk_group(wt, dst, pb, sp)
                    if not split_jit:
                        tasks = [qk(wqh, qth, pb) for pb in range(IL // 128)]
                        tasks += [qk(wkh, kth, pb) for pb in range(IL // 128)]
                        tasks += [v_group(tb)
                                  for tb in range(4 * sp, 4 * sp + 4)]
                        return tasks, []
                    immediate = [qk(wqh, qth, pb) for pb in range(IL // 128)]
                    immediate += [qk(wkh, kth, pb)
                                  for pb in range(1, IL // 128)]
                    # (emit_by_step, task): each MUST be emitted before the
                    # attention step that consumes its output — the Tile
                    # dependency tracker follows program order, so a
                    # producer emitted after its consumer is a race
                    deferred = [
                        (2, qk(wkh, kth, 0)),
                        (4, v_group(4 * sp)), (6, v_group(4 * sp + 1)),
                        (8, v_group(4 * sp + 2)), (10, v_group(4 * sp + 3)),
                    ]
                    return immediate, deferred

                def attn_step(hp, qs, kb, o_ps, nkb):
                    off = kb * 128 - qs * SPAN   # <0 for off-diag
                    lo = max(off, 0)             # first causal query
                    sg = psum.tile([128, 2, SPAN], F32, tag="sT",
                                   bufs=2, name="sg")
                    for i in range(2):
                        nc.tensor.matmul(
                            sg[:, i, lo:SPAN],
                            kth[hp][kb // 4][64 * i:64 * i + 64,
                                             (kb % 4) * 128:
                                             (kb % 4) * 128 + 128],
                            qth[hp][qs][64 * i:64 * i + 64, lo:SPAN],
                            start=True, stop=True,
                        )
                    pt = work.tile([128, 2, SPAN], BF16, tag="pT",
                                   bufs=6, name="pt")
                    nc.scalar.activation(
                        pt[:, :, lo:SPAN], sg[:, :, lo:SPAN],
                        mybir.ActivationFunctionType.Exp, scale=SCALE)
                    if off >= 0:
                        # zero the strictly-upper triangle of the
                        # diagonal 128x128 block for both heads at once
                        nc.vector.tensor_tensor(
                            out=pt[:, :, lo:lo + 128],
                            in0=pt[:, :, lo:lo + 128],
                            in1=tri01[:].broadcast_to([128, 2, 128]),
                            op=mybir.AluOpType.mult,
                        )
                    for i in range(2):
                        nc.tensor.matmul(
                            o_ps[:, i, lo:SPAN],
                            vth[kb][:, 2 * hp + i, :],
                            pt[:, i, lo:SPAN],
                            start=(kb == 0), stop=(kb == nkb - 1),
                        )

                def attn_norm(hp, qs, o_ps, pe_bcast=False):
                    # one copy frees both o_ps PSUM banks for the next hp
                    o_sb = work.tile([DH + 1, 2, SPAN], F32, tag="o_sb",
                                     bufs=2, name="o_sb")
                    nc.vector.tensor_copy(o_sb[:], o_ps[:])
                    r0 = work.tile([1, 2, SPAN], F32, tag="r0",
                                   bufs=2, name="r0")
                    nc.vector.tensor_copy(r0[0:1, :, :],
                                          o_sb[DH:DH + 1, :, :])
                    rcp = work.tile([1, 2, SPAN], F32, tag="rcp",
                                    bufs=2, name="rcp")
                    nc.vector.reciprocal_approx_fast(rcp[:], r0[0:1, :, :])
                    if pe_bcast:
                        # final norm of the kernel: broadcast 1/denom via a
                        # K=1 matmul into the (idle) o_ps slot — shorter
                        # critical chain than the gpsimd partition_broadcast
                        rb_ps = psum.tile([DH, 2, SPAN], F32, tag="oT",
                                          bufs=1, name="rb_ps")
                        for i in range(2):
                            nc.tensor.matmul(rb_ps[:, i, :], ones64[:],
                                             rcp[0:1, i, :],
                                             start=True, stop=True)
                        rb = rb_ps
                    else:
                        rbt = work.tile([64, 2, SPAN], F32, tag="rb",
                                        bufs=2, name="rb")
                        nc.gpsimd.partition_broadcast(rbt[:], rcp[0:1, :, :])
                        rb = rbt
                    for i in range(2):
                        nc.vector.tensor_tensor(
                            out=oth[hp][qs][64 * i:64 * i + 64, :],
                            in0=o_sb[0:DH, i, :], in1=rb[:, i, :],
                            op=mybir.AluOpType.mult,
                        )

                # prologue: projections for span 0
                for t in proj_tasks(0)[0]:
                    t()

                carry = []
                for sp in range(NSP):
                    qs = sp
                    nkb = 4 * qs + 4
                    # independent PE work to weave into attention stalls:
                    # this span's deferred projections (deadline-ordered),
                    # next span's projections + previous span's out-proj
                    deadlined = list(carry)
                    carry = []
                    pending = []
                    if sp + 1 < NSP:
                        imm, carry = proj_tasks(
                            sp + 1, split_jit=(sp + 1 == NSP - 1))
                        pending += imm
                    if sp >= 1:
                        pending += [wo_group(mb, sp - 1)
                                    for mb in range(DM // 128)]
                        pending += [lambda sp=sp: rs_trigger(sp - 1)]
                    nsteps = nkb * (HL // 2)
                    stride = max(1, nsteps // max(1, len(pending)))
                    step = 0
                    for hp in range(HL // 2):
                        o_ps = psum.tile([DH + 1, 2, SPAN], F32, tag="oT",
                                         bufs=1, name="o_ps")
                        for kb in range(nkb):
                            while deadlined and deadlined[0][0] <= step:
                                deadlined.pop(0)[1]()
                            attn_step(hp, qs, kb, o_ps, nkb)
                            step += 1
                            if INTERLEAVE and step % stride == 0 and pending:
                                pending.pop(0)()
                        attn_norm(hp, qs, o_ps,
                                  pe_bcast=(sp == NSP - 1 and
                                            hp == HL // 2 - 1))
                    while deadlined:
                        deadlined.pop(0)[1]()
                    while pending:
                        pending.pop(0)()
                # epilogue: out-projection of the last span
                # epilogue: all rs->out copies happen here, where the
                # collectives for spans 0..NSP-2 have long completed, so no
                # DMA queue ever blocks mid-pipeline on a collective sem
                for mb in range(DM // 128):
                    wo_group(mb, NSP - 1)()
                rs_trigger(NSP - 1)
                for sp in range(NSP):
                    rs_out(sp)

    nc.compile()
    return nc


_program_cache = None


def make_in_maps(inputs):
    from ml_dtypes import bfloat16

    x = np.asarray(inputs["x"], dtype=np.float32)
    Wq = np.asarray(inputs["Wq"], dtype=np.float32)
    Wkv = np.asarray(inputs["Wkv"], dtype=np.float32)
    Wo = np.asarray(inputs["Wo"], dtype=np.float32)
    bo = np.asarray(inputs["bo"], dtype=np.float32)
    xTb = [np.ascontiguousarray(x[b].T).astype(bfloat16) for b in range(B)]
    wqb = [np.ascontiguousarray(Wq[:, g * IL:(g + 1) * IL]).astype(bfloat16)
           for g in range(2)]
    wkb = [np.ascontiguousarray(
               Wkv[:, g * IL:(g + 1) * IL]).astype(bfloat16)
           for g in range(2)]
    wvb = [np.ascontiguousarray(
               Wkv[:, DM + g * IL:DM + (g + 1) * IL]).astype(bfloat16)
           for g in range(2)]
    wob = [np.ascontiguousarray(Wo[g * IL:(g + 1) * IL, :]).astype(bfloat16)
           for g in range(2)]
    bias0 = bo.reshape(DM, 1)
    bias1 = np.zeros_like(bias0)
    in_maps = []
    for c in range(NCORES):
        b, g = c // 2, c % 2
        in_maps.append({
            "xT": xTb[b],
            "wq": wqb[g],
            "wk": wkb[g],
            "wv": wvb[g],
            "wo": wob[g],
            "bias": bias0 if g == 0 else bias1,
        })
    return in_maps


def kernel(x, Wq, Wkv, Wo, bo):
    global _program_cache
    if _program_cache is None:
        _program_cache = build_program()
    nc = _program_cache

    in_maps = make_in_maps(dict(x=x, Wq=Wq, Wkv=Wkv, Wo=Wo, bo=bo))
    res = run_bass_kernel_spmd(nc, in_maps, list(range(NCORES)))

    out = np.empty((B, N, DM), dtype=np.float32)
    for b in range(B):
        top = res.results[2 * b]["out"]       # dmodel rows 0:512 (bf16)
        bot = res.results[2 * b + 1]["out"]   # dmodel rows 512:1024 (bf16)
        out[b] = np.concatenate([top, bot], axis=0).T.astype(np.float32)
    return out


# revision 34
# speedup vs baseline: 1.0277x; 1.0041x over previous
"""Causal multi-head attention (B=4, N=2048, D=1024, H=16, Dh=64) on 8 TRN2 cores.

Sharding: core c handles batch b=c//2 and head-group g=c%2 (8 of 16 heads).
Megatron-style: Wq/Wkv column-parallel, Wo row-parallel; the per-pair partial
outputs are combined with a ReduceScatter(add) over core pairs {2b, 2b+1}.

Everything on-device runs in a transposed layout ([feature, token]) so that no
PE transposes are needed anywhere:
  Qt/Kt = W-stationary matmuls of xT            -> [inner, tok]
  S^T   = Kt-stationary, Qt-moving              -> [key, query]  (2 heads row-packed,
                                                   concurrently via PE row-tiling)
  P^T   = exp(scale*S^T) via ACT, 0/1-masked    -> [key, query]  bf16
  O^T   = V'-stationary ([V | ones]), P^T-moving-> [65, query]   (row 64 = softmax denom)
  out^T = Wo-stationary, O^T-moving             -> [dmodel, tok]
The host transposes x and pre-casts x/W to bf16; partial outputs travel as
bf16 through the ReduceScatter and are upcast to f32 on the way out.

The pipeline is interleaved per query-span so projections, attention, and the
output projection of successive spans overlap, keeping the PE array dense
(HAM stays at K=8/8).
"""

import sys

sys.path.insert(0, "/opt/trn_rl_repo")

import numpy as np

import concourse.bass as bass
import concourse.mybir as mybir
from concourse import bacc, tile
from concourse.bass_utils import run_bass_kernel_spmd

F32 = mybir.dt.float32
BF16 = mybir.dt.bfloat16

B = 4
N = 2048
DM = 1024          # d_model
H = 16
DH = 64
HL = 8             # local heads per core
IL = HL * DH       # 512, local inner dim
SCALE = DH ** -0.5
SPAN = 512         # query-span / matmul moving size
NSP = N // SPAN    # 4
NKB = N // 128     # 16 key/token blocks
NCORES = 8
INTERLEAVE = True


def build_program(repeat=1, for_sim=False):
    nc = bacc.Bacc("TRN2", target_bir_lowering=False, debug=False,
                   num_devices=1 if for_sim else NCORES)

    xT_d = nc.dram_tensor("xT", [DM, N], BF16, kind="ExternalInput").ap()
    wq_d = nc.dram_tensor("wq", [DM, IL], BF16, kind="ExternalInput").ap()
    wk_d = nc.dram_tensor("wk", [DM, IL], BF16, kind="ExternalInput").ap()
    wv_d = nc.dram_tensor("wv", [DM, IL], BF16, kind="ExternalInput").ap()
    wo_d = nc.dram_tensor("wo", [IL, DM], BF16, kind="ExternalInput").ap()
    bias_d = nc.dram_tensor("bias", [DM, 1], F32, kind="ExternalInput").ap()
    out_d = nc.dram_tensor("out", [DM // 2, N], BF16, kind="ExternalOutput").ap()

    with tile.TileContext(nc) as tc:
        with (
            tc.tile_pool(name="weights", bufs=1) as wpool,
            tc.tile_pool(name="acts", bufs=1) as apool,
            tc.tile_pool(name="work", bufs=3) as work,
            tc.tile_pool(name="psum", bufs=1, space="PSUM") as psum,
            tc.tile_pool(name="dram", bufs=1, space="DRAM") as dram,
        ):
            # ---------- stage 0: load (already bf16 from host) ----------
            xh = [apool.tile([128, N], BF16, name=f"xh{pb}", tag=f"xh{pb}")
                  for pb in range(DM // 128)]

            def load_w(src, n_pb, ncols, nm):
                # one big DMA into a [128, n_pb, ncols] tile; per-block
                # [128, ncols] views are sliced out for the matmuls
                t = wpool.tile([128, n_pb, ncols], BF16, name=nm, tag=nm)
                nc.sync.dma_start(
                    t[:], src[:].rearrange("(b p) c -> p b c", p=128))
                return [t[:, pb, :] for pb in range(n_pb)]

            # interleave x-span0 / wq blocks so the first projection's
            # accumulation chain starts after only a few transfers
            wqh_t = wpool.tile([128, DM // 128, IL], BF16, name="wqh",
                               tag="wqh")
            wqh = [wqh_t[:, pb, :] for pb in range(DM // 128)]
            for pb in range(DM // 128):
                nc.sync.dma_start(xh[pb][:, 0:SPAN],
                                  xT_d[pb * 128:(pb + 1) * 128, 0:SPAN])
                nc.sync.dma_start(wqh_t[:, pb, :],
                                  wq_d[pb * 128:(pb + 1) * 128, :])
            wkh = load_w(wk_d, DM // 128, IL, "wkh")
            wvh = load_w(wv_d, DM // 128, IL, "wvh")
            # rest of x in one bulk DMA per partition block
            for pb in range(DM // 128):
                nc.sync.dma_start(xh[pb][:, SPAN:N],
                                  xT_d[pb * 128:(pb + 1) * 128, SPAN:N])
            woh = load_w(wo_d, IL // 128, DM, "woh")

            bias_sb = wpool.tile([128, DM // 128], F32, name="bias_sb")
            for mb in range(DM // 128):
                nc.sync.dma_start(bias_sb[:, mb:mb + 1],
                                  bias_d[mb * 128:(mb + 1) * 128, :])

            # 0/1 lower-triangle mask (keep query >= key within a diag block)
            tri_f = work.tile([128, 128], F32, tag="tri_f", bufs=1)
            nc.gpsimd.memset(tri_f[:], 1.0)
            nc.gpsimd.affine_select(
                out=tri_f[:], in_=tri_f[:],
                compare_op=mybir.AluOpType.is_ge,
                fill=0.0, base=0, channel_multiplier=-1,
                pattern=[[1, 128]],
            )
            tri01 = wpool.tile([128, 1, 128], BF16, name="tri01")
            nc.vector.tensor_copy(tri01[:, 0, :], tri_f[:])
            ones64 = wpool.tile([1, DH], F32, name="ones64")
            nc.vector.memset(ones64[:], 1.0)

            # PE warm-up: ~6us of dummy matmuls overlapping the input DMAs
            # so the HAM clock gate is at K=8/8 when the first real
            # projection matmul issues (cold PE runs at half rate)
            warm_ps = psum.tile([128, 128], F32, tag="projrb", bufs=2,
                                name="warm_ps")
            for _w in range(60):
                nc.tensor.matmul(warm_ps[:], tri01[:, 0, :], tri01[:, 0, :],
                                 start=True, stop=True)

            for _rep in range(repeat):
                # per-span activation tiles
                qth = [[apool.tile([128, SPAN], BF16, name=f"qt{pb}_{sp}",
                                   tag=f"qt{pb}_{sp}")
                        for sp in range(NSP)] for pb in range(IL // 128)]
                kth = [[apool.tile([128, SPAN], BF16, name=f"kt{pb}_{sp}",
                                   tag=f"kt{pb}_{sp}")
                        for sp in range(NSP)] for pb in range(IL // 128)]
                vth = [apool.tile([128, HL, DH + 1], BF16, name=f"vt{tb}",
                                  tag=f"vt{tb}") for tb in range(NKB)]
                oth = [[apool.tile([128, SPAN], BF16, name=f"ot{pb}_{sp}",
                                   tag=f"ot{pb}_{sp}")
                        for sp in range(NSP)] for pb in range(IL // 128)]
                parts = [dram.tile([DM, SPAN], BF16, name=f"part{sp}")
                         for sp in range(NSP)]

                def qk_group(wt, dst, pb, sp):
                    def go():
                        pp = psum.tile([128, SPAN], F32, tag="projrb",
                                       bufs=2, name="pp")
                        for kk in range(DM // 128):
                            nc.tensor.matmul(
                                pp[:],
                                wt[kk][:, pb * 128:(pb + 1) * 128],
                                xh[kk][:, sp * SPAN:(sp + 1) * SPAN],
                                start=(kk == 0), stop=(kk == DM // 128 - 1),
                            )
                        nc.vector.tensor_copy(dst[pb][sp][:], pp[:])
                    return go

                def v_group(tb):
                    def go():
                        pp = psum.tile([128, IL], F32, tag="projrb", bufs=2,
                                       name="ppv")
                        for kk in range(DM // 128):
                            nc.tensor.matmul(
                                pp[:], xh[kk][:, tb * 128:(tb + 1) * 128],
                                wvh[kk][:],
                                start=(kk == 0), stop=(kk == DM // 128 - 1),
                            )
                        nc.vector.tensor_copy(
                            vth[tb][:, :, 0:DH],
                            pp[:].rearrange("p (h d) -> p h d", h=HL))
                        nc.vector.memset(vth[tb][:, :, DH:DH + 1], 1.0)
                    return go

                def wo_group(mb, sp):
                    def go():
                        # the last span's groups borrow the (idle by then)
                        # sT slots so four groups can pre-run their first
                        # three matmuls while the final attn_norm finishes
                        tag = "sT" if sp == NSP - 1 and mb % 2 == 0 \
                            else "projrb"
                        pw = psum.tile([128, SPAN], F32, tag=tag, bufs=2,
                                       name="pw")
                        for ib in range(IL // 128):
                            nc.tensor.matmul(
                                pw[:],
                                woh[ib][:, mb * 128:(mb + 1) * 128],
                                oth[ib][sp][:],
                                start=(ib == 0), stop=(ib == IL // 128 - 1),
                            )
                        po = work.tile([128, SPAN], BF16, tag="po", bufs=3,
                                       name="po")
                        nc.vector.tensor_scalar(
                            out=po[:], in0=pw[:],
                            scalar1=bias_sb[:, mb:mb + 1], scalar2=None,
                            op0=mybir.AluOpType.add,
                        )
                        nc.sync.dma_start(
                            parts[sp][mb * 128:(mb + 1) * 128, :], po[:])
                    return go

                rs_tiles = {}

                def rs_trigger(sp):
                    if for_sim:
                        rs_tiles[sp] = parts[sp]
                        return
                    rs = dram.tile([DM // 2, SPAN], BF16, name=f"rs{sp}")
                    nc.gpsimd.collective_compute(
                        "ReduceScatter", mybir.AluOpType.add,
                        replica_groups=[[0, 1], [2, 3], [4, 5], [6, 7]],
                        ins=[parts[sp].opt()], outs=[rs.opt()],
                    )
                    rs_tiles[sp] = rs

                def rs_out(sp):
                    # issued well after the trigger so the Sync queue never
                    # blocks on the collective semaphore
                    nc.sync.dma_start(out_d[:, sp * SPAN:(sp + 1) * SPAN],
                                      rs_tiles[sp][0:DM // 2, :])



                def proj_tasks(sp, split_jit=False):
                    """Returns (immediate, deferred) task lists. When
                    split_jit, the diagonal key-span tiles (consumed only
                    from step 4*sp of span sp's own attention) are deferred
                    into that span as (emit_by_step, task) pairs, giving
                    the PE filler work while ACT paces the steps. Each
                    deferred task MUST be emitted before the attention step
                    that consumes its output — the Tile dependency tracker
                    follows program order, so a producer emitted after its
                    consumer is a race."""
                    qk = lambda wt, dst, pb: qk_group(wt, dst, pb, sp)
                    if not split_jit:
                        tasks = [qk(wqh, qth, pb) for pb in range(IL // 128)]
                        tasks += [qk(wkh, kth, pb) for pb in range(IL // 128)]
                        tasks += [v_group(tb)
                                  for tb in range(4 * sp, 4 * sp + 4)]
                        return tasks, []
                    immediate = [qk(wqh, qth, pb) for pb in range(IL // 128)]
                    immediate += [qk(wkh, kth, pb)
                                  for pb in range(1, IL // 128)]
                    d0 = 4 * sp  # first consuming step (hp0, kb=4*sp)
                    deferred = [(max(0, d0 - 3), qk(wkh, kth, 0))]
                    deferred += [(max(0, d0 + k - 3), v_group(d0 + k))
                                 for k in range(4)]
                    return immediate, deferred

                def attn_step(hp, qs, kb, o_ps, nkb):
                    off = kb * 128 - qs * SPAN   # <0 for off-diag
                    lo = max(off, 0)             # first causal query
                    sg = psum.tile([128, 2, SPAN], F32, tag="sT",
                                   bufs=2, name="sg")
                    for i in range(2):
                        nc.tensor.matmul(
                            sg[:, i, lo:SPAN],
                            kth[hp][kb // 4][64 * i:64 * i + 64,
                                             (kb % 4) * 128:
                                             (kb % 4) * 128 + 128],
                            qth[hp][qs][64 * i:64 * i + 64, lo:SPAN],
                            start=True, stop=True,
                        )
                    pt = work.tile([128, 2, SPAN], BF16, tag="pT",
                                   bufs=6, name="pt")
                    nc.scalar.activation(
                        pt[:, :, lo:SPAN], sg[:, :, lo:SPAN],
                        mybir.ActivationFunctionType.Exp, scale=SCALE)
                    if off >= 0:
                        # zero the strictly-upper triangle of the
                        # diagonal 128x128 block for both heads at once
                        nc.vector.tensor_tensor(
                            out=pt[:, :, lo:lo + 128],
                            in0=pt[:, :, lo:lo + 128],
                            in1=tri01[:].broadcast_to([128, 2, 128]),
                            op=mybir.AluOpType.mult,
                        )
                    for i in range(2):
                        nc.tensor.matmul(
                            o_ps[:, i, lo:SPAN],
                            vth[kb][:, 2 * hp + i, :],
                            pt[:, i, lo:SPAN],
                            start=(kb == 0), stop=(kb == nkb - 1),
                        )

                def attn_norm(hp, qs, o_ps, pe_bcast=False):
                    # one copy frees both o_ps PSUM banks for the next hp
                    o_sb = work.tile([DH + 1, 2, SPAN], F32, tag="o_sb",
                                     bufs=2, name="o_sb")
                    nc.vector.tensor_copy(o_sb[:], o_ps[:])
                    r0 = work.tile([1, 2, SPAN], F32, tag="r0",
                                   bufs=2, name="r0")
                    nc.vector.tensor_copy(r0[0:1, :, :],
                                          o_sb[DH:DH + 1, :, :])
                    rcp = work.tile([1, 2, SPAN], F32, tag="rcp",
                                    bufs=2, name="rcp")
                    nc.vector.reciprocal_approx_fast(rcp[:], r0[0:1, :, :])
                    if pe_bcast:
                        # final norm of the kernel: broadcast 1/denom via a
                        # K=1 matmul into the (idle) o_ps slot — shorter
                        # critical chain than the gpsimd partition_broadcast
                        rb_ps = psum.tile([DH, 2, SPAN], F32, tag="oT",
                                          bufs=1, name="rb_ps")
                        for i in range(2):
                            nc.tensor.matmul(rb_ps[:, i, :], ones64[:],
                                             rcp[0:1, i, :],
                                             start=True, stop=True)
                        rb = rb_ps
                    else:
                        rbt = work.tile([64, 2, SPAN], F32, tag="rb",
                                        bufs=2, name="rb")
                        nc.gpsimd.partition_broadcast(rbt[:], rcp[0:1, :, :])
                        rb = rbt
                    for i in range(2):
                        nc.vector.tensor_tensor(
                            out=oth[hp][qs][64 * i:64 * i + 64, :],
                            in0=o_sb[0:DH, i, :], in1=rb[:, i, :],
                            op=mybir.AluOpType.mult,
                        )

                # prologue: projections for span 0
                for t in proj_tasks(0)[0]:
                    t()

                carry = []
                for sp in range(NSP):
                    qs = sp
                    nkb = 4 * qs + 4
                    # independent PE work to weave into attention stalls:
                    # this span's deferred projections (deadline-ordered),
                    # next span's projections + previous span's out-proj
                    deadlined = list(carry)
                    carry = []
                    pending = []
                    if sp + 1 < NSP:
                        imm, carry = proj_tasks(sp + 1, split_jit=True)
                        pending += imm
                    if sp >= 1:
                        pending += [wo_group(mb, sp - 1)
                                    for mb in range(DM // 128)]
                        pending += [lambda sp=sp: rs_trigger(sp - 1)]
                    nsteps = nkb * (HL // 2)
                    stride = max(1, nsteps // max(1, len(pending)))
                    step = 0
                    for hp in range(HL // 2):
                        o_ps = psum.tile([DH + 1, 2, SPAN], F32, tag="oT",
                                         bufs=1, name="o_ps")
                        for kb in range(nkb):
                            while deadlined and deadlined[0][0] <= step:
                                deadlined.pop(0)[1]()
                            attn_step(hp, qs, kb, o_ps, nkb)
                            step += 1
                            if INTERLEAVE and step % stride == 0 and pending:
                                pending.pop(0)()
                        attn_norm(hp, qs, o_ps,
                                  pe_bcast=(sp == NSP - 1 and
                                            hp == HL // 2 - 1))
                    while deadlined:
                        deadlined.pop(0)[1]()
                    while pending:
                        pending.pop(0)()
                # epilogue: out-projection of the last span
                # epilogue: all rs->out copies happen here, where the
                # collectives for spans 0..NSP-2 have long completed, so no
                # DMA queue ever blocks mid-pipeline on a collective sem
                for mb in range(DM // 128):
                    wo_group(mb, NSP - 1)()
                rs_trigger(NSP - 1)
                for sp in range(NSP):
                    rs_out(sp)

    nc.compile()
    return nc


_program_cache = None


def make_in_maps(inputs):
    from ml_dtypes import bfloat16

    x = np.asarray(inputs["x"], dtype=np.float32)
    Wq = np.asarray(inputs["Wq"], dtype=np.float32)
    Wkv = np.asarray(inputs["Wkv"], dtype=np.float32)
    Wo = np.asarray(inputs["Wo"], dtype=np.float32)
    bo = np.asarray(inputs["bo"], dtype=np.float32)
    xTb = [np.ascontiguousarray(x[b].T).astype(bfloat16) for b in range(B)]
    wqb = [np.ascontiguousarray(Wq[:, g * IL:(g + 1) * IL]).astype(bfloat16)
           for g in range(2)]
    wkb = [np.ascontiguousarray(
               Wkv[:, g * IL:(g + 1) * IL]).astype(bfloat16)
           for g in range(2)]
    wvb = [np.ascontiguousarray(
               Wkv[:, DM + g * IL:DM + (g + 1) * IL]).astype(bfloat16)
           for g in range(2)]
    wob = [np.ascontiguousarray(Wo[g * IL:(g + 1) * IL, :]).astype(bfloat16)
           for g in range(2)]
    bias0 = bo.reshape(DM, 1)
    bias1 = np.zeros_like(bias0)
    in_maps = []
    for c in range(NCORES):
        b, g = c // 2, c % 2
        in_maps.append({
            "xT": xTb[b],
            "wq": wqb[g],
            "wk": wkb[g],
            "wv": wvb[g],
            "wo": wob[g],
            "bias": bias0 if g == 0 else bias1,
        })
    return in_maps


def kernel(x, Wq, Wkv, Wo, bo):
    global _program_cache
    if _program_cache is None:
        _program_cache = build_program()
    nc = _program_cache

    in_maps = make_in_maps(dict(x=x, Wq=Wq, Wkv=Wkv, Wo=Wo, bo=bo))
    res = run_bass_kernel_spmd(nc, in_maps, list(range(NCORES)))

    out = np.empty((B, N, DM), dtype=np.float32)
    for b in range(B):
        top = res.results[2 * b]["out"]       # dmodel rows 0:512 (bf16)
        bot = res.results[2 * b + 1]["out"]   # dmodel rows 512:1024 (bf16)
        out[b] = np.concatenate([top, bot], axis=0).T.astype(np.float32)
    return out
